# revision 26
# baseline (speedup 1.0000x reference)
"""Causal attention kernel for TRN2, 8 NeuronCores (SPMD).

Problem:  x[4096,2048] f32; q = x@Wq.T, k = x@Wk.T (d_head=128),
          scores = q@k.T causal-masked, attn = softmax(scores),
          out = (attn @ x) @ W2.T.

Sharding: sequence-parallel over queries with stride-8 interleave:
  core c owns queries {8m+c : m=0..511}.  For key tile kt (128 keys),
  every core has exactly 512-16*kt valid queries -- a contiguous tail
  slice of its query columns -- so the SPMD program is identical on all
  cores (no dynamic control flow, no collectives) and causal work is
  perfectly balanced.

Precision: fp16 inputs for the q/k projections and the score matmul
  (fp32 PSUM accumulation), unnormalized softmax (exp without
  max-subtraction: scores are bounded ~|s|<70 for unit-normal inputs,
  safely inside fp32 exp range), attention weights in bf16 (bf16 has
  fp32 exponent range, needed for exp(s) up to ~1e28), V and W2
  matmuls in bf16/fp16, normalization by the softmax row-sum applied
  at the attn_out eviction (keeps fp16 in range).

Scheduling notes (v4):
  * ALL loads go through the sync engine's HWDGE queues in priority
    order (small projection inputs -> xT stream -> xv -> W2).  The 8
    queues are drained round-robin with per-queue FIFO, so issue order
    == byte order; nothing starves the critical path (v2/v3 lost
    ~30us to xv flooding a parallel queue set at t=0).
  * kT / attnT are split into per-keytile tiles so Tile's dependency
    tracking lets scores/exp/V-matmuls pipeline INTO the xT DMA
    stream instead of waiting for a whole-tensor barrier.
  * The V matmul for output chunks 0-3 is fused into the score loop
    (PSUM budget: kT 1 + scores 2 + denom 1 + V 4 = 8 banks); chunks
    4-15 run right after from SBUF-resident attnT.
  * An AllGather-based sharded-kT variant was measured: the 8-core
    0.5MB AllGather costs ~100us on this runtime -- slower than just
    replicating the kT projection (27us compute, overlapped DMA).
"""

from contextlib import ExitStack

import numpy as np
import ml_dtypes

import concourse.bass as bass
import concourse.bacc as bacc
import concourse.mybir as mybir
import concourse.tile as tile
from concourse.bass_utils import run_bass_kernel_spmd
from concourse.tile_rust import add_dep_helper

N_CTX = 4096
D_MODEL = 2048
D_HEAD = 128
NCORES = 8
QPC = N_CTX // NCORES          # 512 queries per core
NKT = N_CTX // 128             # 32 key tiles
NDM = D_MODEL // 128           # 16 d_model chunks
KG = 256                       # kT projection key-group width
NKG = N_CTX // KG
MASK_NEG = -1.0e30

F16 = mybir.dt.float16
BF16 = mybir.dt.bfloat16
F32 = mybir.dt.float32


def _widths():
    # valid query-column width per key tile (tail slice [512-w : 512] of qT)
    return [QPC - 16 * kt for kt in range(NKT)]


def build_program():
    nc = bacc.Bacc(trn_type="TRN2", target_bir_lowering=False, debug=False)

    # ---- DRAM parameters (identical shapes on all cores; data differs) ----
    # xqr[r, 512*ic + m] = x[8m+c, 128*ic + r]   (own-query columns, packed)
    xqr = nc.declare_dram_parameter("xqr", [128, NDM * QPC], F16, isOutput=False)
    # xtp[kg][r, KG*ic + n] = x[KG*kg + n, 128*ic + r]  (contiguous per-kg tiles)
    xtp = nc.declare_dram_parameter("xtp", [NKG, 128, NDM * KG], F16, isOutput=False)
    # xv = x (natural layout), bf16
    xv = nc.declare_dram_parameter("xv", [N_CTX, D_MODEL], BF16, isOutput=False)
    # wqr[r, 128*ic + h] = Wq[h, 128*ic + r]; same for wkr
    wqr = nc.declare_dram_parameter("wqr", [128, D_MODEL], F16, isOutput=False)
    wkr = nc.declare_dram_parameter("wkr", [128, D_MODEL], F16, isOutput=False)
    # w2r[oc][r, 128*ic + o] = W2[128*oc + o, 128*ic + r]
    w2r = nc.declare_dram_parameter("w2r", [NDM, 128, D_MODEL], F16, isOutput=False)
    maskb = nc.declare_dram_parameter("maskb", [128, 16], F32, isOutput=False)
    outT = nc.declare_dram_parameter("outT", [D_MODEL, QPC], F16, isOutput=True)

    W = _widths()

    with tile.TileContext(nc) as tc:
        with (
            tc.tile_pool(name="static", bufs=1) as st,
            tc.tile_pool(name="xvpool", bufs=NKT) as xvp,
            tc.tile_pool(name="ktpool", bufs=NKG) as ktp,
            tc.tile_pool(name="atpool", bufs=1) as atp,
        ):
            qT_sb = st.tile([128, QPC], F16, tag="qT")
            ones_sb = st.tile([128, 1], BF16, tag="ones")
            mask_sb = st.tile([128, 16], F32, tag="mask")
            recip_sb = st.tile([128, QPC], F32, tag="recip")
            nc.vector.memset(ones_sb[:], 1.0)

            # ---- critical small loads first (sync queue order == byte order)
            nc.sync.dma_start(out=mask_sb[:], in_=maskb[:])

            es1 = ExitStack()  # SBUF transients: p1 + xts (freed before p34)
            p1 = es1.enter_context(tc.tile_pool(name="p1", bufs=1))
            wq_sb = p1.tile([128, D_MODEL], F16, tag="wq")
            nc.sync.dma_start(out=wq_sb[:], in_=wqr[:])
            wk_sb = p1.tile([128, D_MODEL], F16, tag="wk")
            nc.sync.dma_start(out=wk_sb[:], in_=wkr[:])
            xq_sb = p1.tile([128, NDM * QPC], F16, tag="xq")
            for qq in range(4):
                nc.sync.dma_start(
                    out=xq_sb[:, 4 * QPC * qq : 4 * QPC * (qq + 1)],
                    in_=xqr[:, 4 * QPC * qq : 4 * QPC * (qq + 1)],
                )

            # ---- xT stream DMAs (one contiguous 1MB DMA per key group) ----
            xts = es1.enter_context(tc.tile_pool(name="xts", bufs=3))
            xts_t = []
            last_xts_dma = None
            for kg in range(NKG):
                t = xts.tile([128, NDM * KG], F16, tag="xts", name=f"xts{kg}")
                last_xts_dma = nc.sync.dma_start(out=t[:], in_=xtp[kg])
                xts_t.append(t)

            # ---- xv loads: hard-ordered AFTER the xT stream.  The sync
            # sequencer blocks issuing xv[0] until the last xT DMA lands, so
            # the 16.8MB xT stream (which gates kT -> scores -> everything)
            # gets the full HBM bandwidth; xv then streams during the
            # xv-paced V-matmul tail.
            xv_t = []
            for kt in range(NKT):
                t = xvp.tile([128, D_MODEL], BF16, tag="xv", name=f"xv{kt}")
                d = nc.sync.dma_start(out=t[:], in_=xv[128 * kt : 128 * (kt + 1), :])
                if kt == 0:
                    add_dep_helper(d.ins, last_xts_dma.ins,
                                   reason="xv bytes strictly after xT stream")
                xv_t.append(t)

            # ---- qT projection ----
            with tc.tile_pool(name="psq", bufs=1, space="PSUM") as psqp:
                psq = psqp.tile([128, QPC], F32, tag="psq")
                for ic in range(NDM):
                    nc.tensor.matmul(
                        psq[:],
                        wq_sb[:, 128 * ic : 128 * (ic + 1)],
                        xq_sb[:, QPC * ic : QPC * (ic + 1)],
                        start=(ic == 0), stop=(ic == NDM - 1),
                    )
                nc.vector.tensor_copy(qT_sb[:], psq[:])

            # ---- fused pipeline: kT proj / scores / exp / denom / V[0:4] ----
            es2 = ExitStack()  # PSUM: psv1 (freed mid-way through V chunk waves)
            psv1 = es2.enter_context(
                tc.tile_pool(name="psv1", bufs=4, space="PSUM", side="right")
            )
            pso1 = [
                psv1.tile([128, QPC], F32, tag="pso1", name=f"pso1_{j}")
                for j in range(4)
            ]
            at_t = []
            with (
                tc.tile_pool(name="psk", bufs=1, space="PSUM") as pskp,
                tc.tile_pool(name="pss", bufs=2, space="PSUM") as pssp,
                tc.tile_pool(name="psd", bufs=1, space="PSUM") as psdp,
            ):
                psd = psdp.tile([1, QPC], F32, tag="psd")
                for kg in range(NKG):
                    psk = pskp.tile([128, KG], F32, tag="psk", name=f"psk{kg}")
                    for ic in range(NDM):
                        nc.tensor.matmul(
                            psk[:],
                            wk_sb[:, 128 * ic : 128 * (ic + 1)],
                            xts_t[kg][:, KG * ic : KG * (ic + 1)],
                            start=(ic == 0), stop=(ic == NDM - 1),
                        )
                    ktile = ktp.tile([128, KG], F16, tag="kt", name=f"kt{kg}")
                    nc.vector.tensor_copy(ktile[:], psk[:])

                    for sub in range(KG // 128):
                        kt = kg * (KG // 128) + sub
                        w = W[kt]
                        ps = pssp.tile([128, 512], F32, tag="pss", name=f"pss{kt}")
                        nc.tensor.matmul(
                            ps[:, :w],
                            ktile[:, 128 * sub : 128 * (sub + 1)],
                            qT_sb[:, QPC - w : QPC],
                            start=True, stop=True,
                        )
                        nc.vector.tensor_add(ps[:, :16], ps[:, :16], mask_sb[:])
                        at = atp.tile([128, w], BF16, tag=f"at{kt}")
                        nc.scalar.activation(
                            at[:], ps[:, :w], mybir.ActivationFunctionType.Exp
                        )
                        at_t.append(at)
                        nc.tensor.matmul(
                            psd[0:1, QPC - w : QPC],
                            ones_sb[:],
                            at[:],
                            start=(kt == 0), stop=(kt == NKT - 1),
                        )
                        # V matmul for output chunks 0-3, fused
                        for j in range(4):
                            nc.tensor.matmul(
                                pso1[j][:, QPC - w : QPC],
                                xv_t[kt][:, 128 * j : 128 * (j + 1)],
                                at[:],
                                start=(kt == 0), stop=(kt == NKT - 1),
                            )

                # softmax denominators -> reciprocal (reads psd before the
                # pool closes; broadcast follows outside)
                nc.vector.reciprocal(recip_sb[0:1, :], psd[0:1, :])

            es1.close()  # free p1 + xts SBUF for the aoT / W2 pools
            nc.gpsimd.partition_broadcast(recip_sb[:], recip_sb[0:1, :])

            with tc.tile_pool(name="p34", bufs=1) as p34:
                ao_t = {}

                def v_group(pool, ocs):
                    pso = {
                        oc: pool.tile([128, QPC], F32, tag="pso2", name=f"pso2_{oc}")
                        for oc in ocs
                    }
                    for kt in range(NKT):
                        w = W[kt]
                        for oc in ocs:
                            nc.tensor.matmul(
                                pso[oc][:, QPC - w : QPC],
                                xv_t[kt][:, 128 * oc : 128 * (oc + 1)],
                                at_t[kt][:],
                                start=(kt == 0), stop=(kt == NKT - 1),
                            )
                    for oc in ocs:
                        t = p34.tile([128, QPC], F16, tag=f"ao{oc}")
                        nc.vector.tensor_mul(t[:], pso[oc][:], recip_sb[:])
                        ao_t[oc] = t

                # ---- V matmul for output chunks 4-15, three kt-major 4-chunk
                # groups on explicitly staged PSUM pools (stack allocator:
                # each pool reuses banks of an already-released pool, so
                # group A depends only on the fused pools' release and runs
                # CONCURRENTLY with the xv-paced tail of V[0:4]). ----
                es3 = ExitStack()
                psv2a = es3.enter_context(
                    tc.tile_pool(name="psv2a", bufs=4, space="PSUM")
                )
                v_group(psv2a, range(4, 8))       # banks of psk/pss/psd

                for j in range(4):
                    t = p34.tile([128, QPC], F16, tag=f"ao{j}")
                    # normalize here so fp16 stays in range
                    nc.vector.tensor_mul(t[:], pso1[j][:], recip_sb[:])
                    ao_t[j] = t
                es2.close()                        # release psv1 (banks 4-7)

                with tc.tile_pool(name="psv2b", bufs=4, space="PSUM", side="right") as psv2b:
                    v_group(psv2b, range(8, 12))   # reuses psv1's banks
                    es3.close()                    # release psv2a
                    with tc.tile_pool(name="psv2c", bufs=4, space="PSUM") as psv2c:
                        v_group(psv2c, range(12, NDM))

                # ---- W2: outT = W2T.T @ attn_outT ----
                with (
                    tc.tile_pool(name="w2s", bufs=4) as w2s,
                    tc.tile_pool(name="outs", bufs=4) as outs,
                    tc.tile_pool(name="ps4", bufs=2, space="PSUM") as ps4,
                ):
                    for oc in range(NDM):
                        tw = w2s.tile([128, D_MODEL], F16, tag="w2")
                        nc.sync.dma_start(out=tw[:], in_=w2r[oc])
                        ps = ps4.tile([128, QPC], F32, tag="ps4")
                        for ic in range(NDM):
                            nc.tensor.matmul(
                                ps[:],
                                tw[:, 128 * ic : 128 * (ic + 1)],
                                ao_t[ic][:],
                                start=(ic == 0), stop=(ic == NDM - 1),
                            )
                        t = outs.tile([128, QPC], F16, tag="out")
                        nc.vector.tensor_copy(t[:], ps[:])
                        nc.sync.dma_start(
                            out=outT[128 * oc : 128 * (oc + 1), :], in_=t[:]
                        )

    nc.compile()
    return nc


def prepare_inputs(x, Wk, Wq, W2):
    """Host-side sharding/layout prep. Returns in_maps for the 8 cores."""
    x = np.asarray(x, dtype=np.float32)
    Wk = np.asarray(Wk, dtype=np.float32)
    Wq = np.asarray(Wq, dtype=np.float32)
    W2 = np.asarray(W2, dtype=np.float32)

    xT16 = np.ascontiguousarray(x.T).astype(np.float16)          # [D, N]
    # xtp[kg, r, KG*ic + n] = xT[128*ic + r, KG*kg + n]
    xtp = np.ascontiguousarray(
        xT16.reshape(NDM, 128, NKG, KG).transpose(2, 1, 0, 3).reshape(NKG, 128, NDM * KG)
    )
    xv16 = x.astype(ml_dtypes.bfloat16)                          # [N, D]

    def pack_chunks(aT, width):
        # aT [D_MODEL, width] -> [128, NDM*width]: out[r, width*ic + c] = aT[128ic+r, c]
        return np.ascontiguousarray(
            aT.reshape(NDM, 128, width).transpose(1, 0, 2).reshape(128, NDM * width)
        )

    wqr = pack_chunks(np.ascontiguousarray(Wq.T).astype(np.float16), D_HEAD)
    wkr = pack_chunks(np.ascontiguousarray(Wk.T).astype(np.float16), D_HEAD)
    # w2r[oc, r, 128*ic + o] = W2T[128ic+r, 128oc+o]
    w2T = np.ascontiguousarray(W2.T).astype(np.float16)
    w2r = np.ascontiguousarray(
        w2T.reshape(NDM, 128, NDM, 128).transpose(2, 1, 0, 3).reshape(NDM, 128, D_MODEL)
    )

    in_maps = []
    for c in range(NCORES):
        xqT = np.ascontiguousarray(x[c::NCORES].T).astype(np.float16)  # [D, QPC]
        xqr_c = pack_chunks(xqT, QPC)
        mask = np.zeros((128, 16), dtype=np.float32)
        j = np.arange(128)[:, None]
        t = np.arange(16)[None, :]
        mask[j > 8 * t + c] = MASK_NEG
        in_maps.append(
            {
                "xqr": xqr_c,
                "xtp": xtp,
                "xv": xv16,
                "wqr": wqr,
                "wkr": wkr,
                "w2r": w2r,
                "maskb": mask,
            }
        )
    return in_maps


def assemble_output(results):
    res = np.stack([np.asarray(results[c]["outT"]).astype(np.float32) for c in range(NCORES)])
    # [c, d, m] -> out[8m+c, d]
    return np.ascontiguousarray(res.transpose(2, 0, 1).reshape(N_CTX, D_MODEL))


_CACHED = {}


def kernel(x, Wk, Wq, W2, _trace=False):
    if "nc" not in _CACHED:
        _CACHED["nc"] = build_program()
    nc = _CACHED["nc"]
    in_maps = prepare_inputs(x, Wk, Wq, W2)
    res = run_bass_kernel_spmd(nc, in_maps, core_ids=list(range(NCORES)), trace=_trace)
    out = assemble_output(res.results)
    if _trace:
        return out, res
    return out


# revision 30
# speedup vs baseline: 1.0031x; 1.0031x over previous
"""Causal attention kernel for TRN2, 8 NeuronCores (SPMD).

Problem:  x[4096,2048] f32; q = x@Wq.T, k = x@Wk.T (d_head=128),
          scores = q@k.T causal-masked, attn = softmax(scores),
          out = (attn @ x) @ W2.T.

Sharding: sequence-parallel over queries with stride-8 interleave:
  core c owns queries {8m+c : m=0..511}.  For key tile kt (128 keys),
  every core has exactly 512-16*kt valid queries -- a contiguous tail
  slice of its query columns -- so the SPMD program is identical on all
  cores (no dynamic control flow, no collectives) and causal work is
  perfectly balanced.

Precision: fp16 inputs for the q/k projections and the score matmul
  (fp32 PSUM accumulation), unnormalized softmax (exp without
  max-subtraction: scores are bounded ~|s|<70 for unit-normal inputs,
  safely inside fp32 exp range), attention weights in bf16 (bf16 has
  fp32 exponent range, needed for exp(s) up to ~1e28), V and W2
  matmuls in bf16/fp16, normalization by the softmax row-sum applied
  at the attn_out eviction (keeps fp16 in range).

Scheduling notes (final, ~225us on HW):
  * All input streams are host-packed so every consumer group loads
    with one large contiguous DMA (HWDGE issue costs ~0.6us each; a
    naive per-tile version with 481 issues was issue-bound at 457us).
  * Byte schedule: small projection inputs -> xtp (packed x^T, pacing
    kT -> scores -> exp) -> xv (x natural, pacing the V matmul) -> w2r.
    xv is hard-ordered after the xT stream via an explicit dependency
    on the issuing sequencer: the machine is DMA-bound at ~270 GB/s/core
    through the first two streams, so any byte stealing from xtp slows
    the critical path 1:1 (measured both ways).
  * kT / attnT are split into per-keytile tiles so Tile's dependency
    tracking lets scores/exp/denominator/V pipeline into the DMA
    streams with no whole-tensor barriers.
  * The V matmul for output chunks 0-3 is fused into the score loop
    (PSUM: kT 2 + scores 1 + denom 1 + V 4 = 8 banks); chunks 4-15 run
    as kt-major groups on explicitly staged PSUM pools (stack
    allocator: group A reuses the fused pools' banks and overlaps the
    xv-paced tail of V[0:3]; B/C follow as banks release).  V
    accumulation groups execute start->stop in kt order, so xv must
    stream ascending.
  * Rejected variants (measured): AllGather-sharded kT (8-core 0.5MB
    AllGather costs ~100us here, replication is cheaper), interleaving
    xv into the xtp stream (fused phase is DMA-paced, no slack),
    descending xv (PSUM accumulation groups execute in emission order,
    the whole group waited for xv[0]).
"""

from contextlib import ExitStack

import numpy as np
import ml_dtypes

import concourse.bass as bass
import concourse.bacc as bacc
import concourse.mybir as mybir
import concourse.tile as tile
from concourse.bass_utils import run_bass_kernel_spmd
from concourse.tile_rust import add_dep_helper

N_CTX = 4096
D_MODEL = 2048
D_HEAD = 128
NCORES = 8
QPC = N_CTX // NCORES          # 512 queries per core
NKT = N_CTX // 128             # 32 key tiles
NDM = D_MODEL // 128           # 16 d_model chunks
KG = 256                       # kT projection key-group width
NKG = N_CTX // KG
MASK_NEG = -1.0e30

F16 = mybir.dt.float16
BF16 = mybir.dt.bfloat16
F32 = mybir.dt.float32


def _widths():
    # valid query-column width per key tile (tail slice [512-w : 512] of qT)
    return [QPC - 16 * kt for kt in range(NKT)]


def build_program():
    nc = bacc.Bacc(trn_type="TRN2", target_bir_lowering=False, debug=False)

    # ---- DRAM parameters (identical shapes on all cores; data differs) ----
    # xqr[r, 512*ic + m] = x[8m+c, 128*ic + r]   (own-query columns, packed)
    xqr = nc.declare_dram_parameter("xqr", [128, NDM * QPC], F16, isOutput=False)
    # xtp[kg][r, KG*ic + n] = x[KG*kg + n, 128*ic + r]  (contiguous per-kg tiles)
    xtp = nc.declare_dram_parameter("xtp", [NKG, 128, NDM * KG], F16, isOutput=False)
    # xv = x (natural layout), bf16
    xv = nc.declare_dram_parameter("xv", [N_CTX, D_MODEL], BF16, isOutput=False)
    # wqr[r, 128*ic + h] = Wq[h, 128*ic + r]; same for wkr
    wqr = nc.declare_dram_parameter("wqr", [128, D_MODEL], F16, isOutput=False)
    wkr = nc.declare_dram_parameter("wkr", [128, D_MODEL], F16, isOutput=False)
    # w2r[oc][r, 128*ic + o] = W2[128*oc + o, 128*ic + r]
    w2r = nc.declare_dram_parameter("w2r", [NDM, 128, D_MODEL], F16, isOutput=False)
    maskb = nc.declare_dram_parameter("maskb", [128, 16], F32, isOutput=False)
    outT = nc.declare_dram_parameter("outT", [D_MODEL, QPC], F16, isOutput=True)

    W = _widths()

    with tile.TileContext(nc) as tc:
        with (
            tc.tile_pool(name="static", bufs=1) as st,
            tc.tile_pool(name="xvpool", bufs=NKT) as xvp,
            tc.tile_pool(name="ktpool", bufs=NKG) as ktp,
            tc.tile_pool(name="atpool", bufs=1) as atp,
        ):
            qT_sb = st.tile([128, QPC], F16, tag="qT")
            ones_sb = st.tile([128, 1], BF16, tag="ones")
            mask_sb = st.tile([128, 16], F32, tag="mask")
            recip_sb = st.tile([128, QPC], F32, tag="recip")
            nc.vector.memset(ones_sb[:], 1.0)

            # ---- critical small loads first (sync queue order == byte order)
            nc.sync.dma_start(out=mask_sb[:], in_=maskb[:])

            es1 = ExitStack()  # SBUF transients: p1 + xts (freed before p34)
            p1 = es1.enter_context(tc.tile_pool(name="p1", bufs=1))
            wq_sb = p1.tile([128, D_MODEL], F16, tag="wq")
            nc.sync.dma_start(out=wq_sb[:], in_=wqr[:])
            wk_sb = p1.tile([128, D_MODEL], F16, tag="wk")
            nc.sync.dma_start(out=wk_sb[:], in_=wkr[:])
            xq_sb = p1.tile([128, NDM * QPC], F16, tag="xq")
            for qq in range(4):
                nc.sync.dma_start(
                    out=xq_sb[:, 4 * QPC * qq : 4 * QPC * (qq + 1)],
                    in_=xqr[:, 4 * QPC * qq : 4 * QPC * (qq + 1)],
                )

            # ---- xT stream DMAs (one contiguous 1MB DMA per key group) ----
            xts = es1.enter_context(tc.tile_pool(name="xts", bufs=3))
            xts_t = []
            last_xts_dma = None
            for kg in range(NKG):
                t = xts.tile([128, NDM * KG], F16, tag="xts", name=f"xts{kg}")
                last_xts_dma = nc.sync.dma_start(out=t[:], in_=xtp[kg])
                xts_t.append(t)

            # ---- xv loads: hard-ordered AFTER the xT stream (the fused
            # phase is DMA-paced on xtp; sharing bandwidth with xv slows the
            # critical stream 1:1 -- measured) ----
            xv_t = []
            for kt in range(NKT):
                t = xvp.tile([128, D_MODEL], BF16, tag="xv", name=f"xv{kt}")
                d = nc.sync.dma_start(out=t[:], in_=xv[128 * kt : 128 * (kt + 1), :])
                if kt == 0:
                    add_dep_helper(d.ins, last_xts_dma.ins,
                                   reason="xv bytes strictly after xT stream")
                xv_t.append(t)

            # ---- qT projection ----
            with tc.tile_pool(name="psq", bufs=1, space="PSUM") as psqp:
                psq = psqp.tile([128, QPC], F32, tag="psq")
                for ic in range(NDM):
                    nc.tensor.matmul(
                        psq[:],
                        wq_sb[:, 128 * ic : 128 * (ic + 1)],
                        xq_sb[:, QPC * ic : QPC * (ic + 1)],
                        start=(ic == 0), stop=(ic == NDM - 1),
                    )
                nc.vector.tensor_copy(qT_sb[:], psq[:])

            # ---- fused pipeline: kT proj / scores / exp / denom / V[0:4] ----
            es2 = ExitStack()  # PSUM: psv1 (freed mid-way through V chunk waves)
            psv1 = es2.enter_context(
                tc.tile_pool(name="psv1", bufs=4, space="PSUM", side="right")
            )
            pso1 = [
                psv1.tile([128, QPC], F32, tag="pso1", name=f"pso1_{j}")
                for j in range(4)
            ]
            at_t = []
            with (
                tc.tile_pool(name="psk", bufs=2, space="PSUM") as pskp,
                tc.tile_pool(name="pss", bufs=1, space="PSUM") as pssp,
                tc.tile_pool(name="psd", bufs=1, space="PSUM") as psdp,
            ):
                psd = psdp.tile([1, QPC], F32, tag="psd")
                for kg in range(NKG):
                    psk = pskp.tile([128, KG], F32, tag="psk", name=f"psk{kg}")
                    for ic in range(NDM):
                        nc.tensor.matmul(
                            psk[:],
                            wk_sb[:, 128 * ic : 128 * (ic + 1)],
                            xts_t[kg][:, KG * ic : KG * (ic + 1)],
                            start=(ic == 0), stop=(ic == NDM - 1),
                        )
                    ktile = ktp.tile([128, KG], F16, tag="kt", name=f"kt{kg}")
                    nc.vector.tensor_copy(ktile[:], psk[:])

                    for sub in range(KG // 128):
                        kt = kg * (KG // 128) + sub
                        w = W[kt]
                        ps = pssp.tile([128, 512], F32, tag="pss", name=f"pss{kt}")
                        nc.tensor.matmul(
                            ps[:, :w],
                            ktile[:, 128 * sub : 128 * (sub + 1)],
                            qT_sb[:, QPC - w : QPC],
                            start=True, stop=True,
                        )
                        nc.vector.tensor_add(ps[:, :16], ps[:, :16], mask_sb[:])
                        at = atp.tile([128, w], BF16, tag=f"at{kt}")
                        nc.scalar.activation(
                            at[:], ps[:, :w], mybir.ActivationFunctionType.Exp
                        )
                        at_t.append(at)
                        nc.tensor.matmul(
                            psd[0:1, QPC - w : QPC],
                            ones_sb[:],
                            at[:],
                            start=(kt == 0), stop=(kt == NKT - 1),
                        )
                        # V matmul for output chunks 0-3, fused
                        for j in range(4):
                            nc.tensor.matmul(
                                pso1[j][:, QPC - w : QPC],
                                xv_t[kt][:, 128 * j : 128 * (j + 1)],
                                at[:],
                                start=(kt == 0), stop=(kt == NKT - 1),
                            )

                # softmax denominators -> reciprocal (reads psd before the
                # pool closes; broadcast follows outside)
                nc.vector.reciprocal(recip_sb[0:1, :], psd[0:1, :])

            es1.close()  # free p1 + xts SBUF for the aoT / W2 pools
            nc.gpsimd.partition_broadcast(recip_sb[:], recip_sb[0:1, :])

            with tc.tile_pool(name="p34", bufs=1) as p34:
                ao_t = {}

                def v_group(pool, ocs):
                    pso = {
                        oc: pool.tile([128, QPC], F32, tag="pso2", name=f"pso2_{oc}")
                        for oc in ocs
                    }
                    for kt in range(NKT):
                        w = W[kt]
                        for oc in ocs:
                            nc.tensor.matmul(
                                pso[oc][:, QPC - w : QPC],
                                xv_t[kt][:, 128 * oc : 128 * (oc + 1)],
                                at_t[kt][:],
                                start=(kt == 0), stop=(kt == NKT - 1),
                            )
                    for oc in ocs:
                        t = p34.tile([128, QPC], F16, tag=f"ao{oc}")
                        nc.vector.tensor_mul(t[:], pso[oc][:], recip_sb[:])
                        ao_t[oc] = t

                # ---- V matmul for output chunks 4-15, three kt-major 4-chunk
                # groups on explicitly staged PSUM pools (stack allocator:
                # each pool reuses banks of an already-released pool, so
                # group A depends only on the fused pools' release and runs
                # CONCURRENTLY with the xv-paced tail of V[0:4]). ----
                es3 = ExitStack()
                psv2a = es3.enter_context(
                    tc.tile_pool(name="psv2a", bufs=4, space="PSUM")
                )
                v_group(psv2a, range(4, 8))       # banks of psk/pss/psd

                for j in range(4):
                    t = p34.tile([128, QPC], F16, tag=f"ao{j}")
                    # normalize here so fp16 stays in range
                    nc.vector.tensor_mul(t[:], pso1[j][:], recip_sb[:])
                    ao_t[j] = t
                es2.close()                        # release psv1 (banks 4-7)

                with tc.tile_pool(name="psv2b", bufs=4, space="PSUM", side="right") as psv2b:
                    v_group(psv2b, range(8, 12))   # reuses psv1's banks
                    es3.close()                    # release psv2a
                    with tc.tile_pool(name="psv2c", bufs=4, space="PSUM") as psv2c:
                        v_group(psv2c, range(12, NDM))

                # ---- W2: outT = W2T.T @ attn_outT ----
                with (
                    tc.tile_pool(name="w2s", bufs=4) as w2s,
                    tc.tile_pool(name="outs", bufs=4) as outs,
                    tc.tile_pool(name="ps4", bufs=2, space="PSUM") as ps4,
                ):
                    for oc in range(NDM):
                        tw = w2s.tile([128, D_MODEL], F16, tag="w2")
                        nc.sync.dma_start(out=tw[:], in_=w2r[oc])
                        ps = ps4.tile([128, QPC], F32, tag="ps4")
                        for ic in range(NDM):
                            nc.tensor.matmul(
                                ps[:],
                                tw[:, 128 * ic : 128 * (ic + 1)],
                                ao_t[ic][:],
                                start=(ic == 0), stop=(ic == NDM - 1),
                            )
                        t = outs.tile([128, QPC], F16, tag="out")
                        nc.vector.tensor_copy(t[:], ps[:])
                        nc.scalar.dma_start(
                            out=outT[128 * oc : 128 * (oc + 1), :], in_=t[:]
                        )

    nc.compile()
    return nc


def prepare_inputs(x, Wk, Wq, W2):
    """Host-side sharding/layout prep. Returns in_maps for the 8 cores."""
    x = np.asarray(x, dtype=np.float32)
    Wk = np.asarray(Wk, dtype=np.float32)
    Wq = np.asarray(Wq, dtype=np.float32)
    W2 = np.asarray(W2, dtype=np.float32)

    xT16 = np.ascontiguousarray(x.T).astype(np.float16)          # [D, N]
    # xtp[kg, r, KG*ic + n] = xT[128*ic + r, KG*kg + n]
    xtp = np.ascontiguousarray(
        xT16.reshape(NDM, 128, NKG, KG).transpose(2, 1, 0, 3).reshape(NKG, 128, NDM * KG)
    )
    xv16 = x.astype(ml_dtypes.bfloat16)                          # [N, D]

    def pack_chunks(aT, width):
        # aT [D_MODEL, width] -> [128, NDM*width]: out[r, width*ic + c] = aT[128ic+r, c]
        return np.ascontiguousarray(
            aT.reshape(NDM, 128, width).transpose(1, 0, 2).reshape(128, NDM * width)
        )

    wqr = pack_chunks(np.ascontiguousarray(Wq.T).astype(np.float16), D_HEAD)
    wkr = pack_chunks(np.ascontiguousarray(Wk.T).astype(np.float16), D_HEAD)
    # w2r[oc, r, 128*ic + o] = W2T[128ic+r, 128oc+o]
    w2T = np.ascontiguousarray(W2.T).astype(np.float16)
    w2r = np.ascontiguousarray(
        w2T.reshape(NDM, 128, NDM, 128).transpose(2, 1, 0, 3).reshape(NDM, 128, D_MODEL)
    )

    in_maps = []
    for c in range(NCORES):
        xqT = np.ascontiguousarray(x[c::NCORES].T).astype(np.float16)  # [D, QPC]
        xqr_c = pack_chunks(xqT, QPC)
        mask = np.zeros((128, 16), dtype=np.float32)
        j = np.arange(128)[:, None]
        t = np.arange(16)[None, :]
        mask[j > 8 * t + c] = MASK_NEG
        in_maps.append(
            {
                "xqr": xqr_c,
                "xtp": xtp,
                "xv": xv16,
                "wqr": wqr,
                "wkr": wkr,
                "w2r": w2r,
                "maskb": mask,
            }
        )
    return in_maps


def assemble_output(results):
    res = np.stack([np.asarray(results[c]["outT"]).astype(np.float32) for c in range(NCORES)])
    # [c, d, m] -> out[8m+c, d]
    return np.ascontiguousarray(res.transpose(2, 0, 1).reshape(N_CTX, D_MODEL))


_CACHED = {}


def kernel(x, Wk, Wq, W2, _trace=False):
    if "nc" not in _CACHED:
        _CACHED["nc"] = build_program()
    nc = _CACHED["nc"]
    in_maps = prepare_inputs(x, Wk, Wq, W2)
    res = run_bass_kernel_spmd(nc, in_maps, core_ids=list(range(NCORES)), trace=_trace)
    out = assemble_output(res.results)
    if _trace:
        return out, res
    return out


# revision 33
# speedup vs baseline: 1.0375x; 1.0343x over previous
"""Causal attention kernel for TRN2, 8 NeuronCores (SPMD).

Problem:  x[4096,2048] f32; q = x@Wq.T, k = x@Wk.T (d_head=128),
          scores = q@k.T causal-masked, attn = softmax(scores),
          out = (attn @ x) @ W2.T.

Sharding: sequence-parallel over queries with stride-8 interleave:
  core c owns queries {8m+c : m=0..511}.  For key tile kt (128 keys),
  every core has exactly 512-16*kt valid queries -- a contiguous tail
  slice of its query columns -- so the SPMD program is identical on all
  cores (no dynamic control flow, no collectives) and causal work is
  perfectly balanced.

Precision: fp16 inputs for the q/k projections and the score matmul
  (fp32 PSUM accumulation), unnormalized softmax (exp without
  max-subtraction: scores are bounded ~|s|<70 for unit-normal inputs,
  safely inside fp32 exp range), attention weights in bf16 (bf16 has
  fp32 exponent range, needed for exp(s) up to ~1e28), V and W2
  matmuls in bf16/fp16, normalization by the softmax row-sum applied
  at the attn_out eviction (keeps fp16 in range).

Scheduling notes (final, ~226us median on HW; 8-core uniform 221-232us):
  * All input streams are host-packed so every consumer group loads
    with one large contiguous DMA (HWDGE issue costs ~0.6us each; a
    naive per-tile version with 481 issues was issue-bound at 457us).
  * Byte schedule: small projection inputs -> xtp (packed x^T, pacing
    kT -> scores -> exp) -> xv (x natural, pacing the V matmul) -> w2r.
    xv is hard-ordered after the xT stream via an explicit dependency
    on the issuing sequencer: the machine is DMA-bound at ~270 GB/s/core
    through the first two streams, so any byte stealing from xtp slows
    the critical path 1:1 (measured both ways).
  * kT / attnT are split into per-keytile tiles so Tile's dependency
    tracking lets scores/exp/denominator/V pipeline into the DMA
    streams with no whole-tensor barriers.
  * The V matmul for output chunks 0-3 is fused into the score loop
    (PSUM: kT 2 + scores 1 + denom 1 + V 4 = 8 banks); chunks 4-15 run
    as kt-major groups on explicitly staged PSUM pools (stack
    allocator: group A reuses the fused pools' banks and overlaps the
    xv-paced tail of V[0:3]; B/C follow as banks release).  V
    accumulation groups execute start->stop in kt order, so xv must
    stream ascending.
  * Rejected variants (measured): AllGather-sharded kT (8-core 0.5MB
    AllGather costs ~100us here, replication is cheaper), interleaving
    xv into the xtp stream (fused phase is DMA-paced, no slack),
    descending xv (PSUM accumulation groups execute in emission order,
    the whole group waited for xv[0]).
"""

from contextlib import ExitStack

import numpy as np
import ml_dtypes

import concourse.bass as bass
import concourse.bacc as bacc
import concourse.mybir as mybir
import concourse.tile as tile
from concourse.bass_utils import run_bass_kernel_spmd
from concourse.tile_rust import add_dep_helper

N_CTX = 4096
D_MODEL = 2048
D_HEAD = 128
NCORES = 8
QPC = N_CTX // NCORES          # 512 queries per core
NKT = N_CTX // 128             # 32 key tiles
NDM = D_MODEL // 128           # 16 d_model chunks
KG = 128                       # kT projection key-group width
NKG = N_CTX // KG
MASK_NEG = -1.0e30

F16 = mybir.dt.float16
BF16 = mybir.dt.bfloat16
F32 = mybir.dt.float32


def _widths():
    # valid query-column width per key tile (tail slice [512-w : 512] of qT)
    return [QPC - 16 * kt for kt in range(NKT)]


def build_program():
    nc = bacc.Bacc(trn_type="TRN2", target_bir_lowering=False, debug=False)

    # ---- DRAM parameters (identical shapes on all cores; data differs) ----
    # xqr[r, 512*ic + m] = x[8m+c, 128*ic + r]   (own-query columns, packed)
    xqr = nc.declare_dram_parameter("xqr", [128, NDM * QPC], F16, isOutput=False)
    # xtp[kg][r, KG*ic + n] = x[KG*kg + n, 128*ic + r]  (contiguous per-kg tiles)
    xtp = nc.declare_dram_parameter("xtp", [NKG, 128, NDM * KG], F16, isOutput=False)
    # xv = x (natural layout), bf16
    xv = nc.declare_dram_parameter("xv", [N_CTX, D_MODEL], BF16, isOutput=False)
    # wqr[r, 128*ic + h] = Wq[h, 128*ic + r]; same for wkr
    wqr = nc.declare_dram_parameter("wqr", [128, D_MODEL], F16, isOutput=False)
    wkr = nc.declare_dram_parameter("wkr", [128, D_MODEL], F16, isOutput=False)
    # w2r[oc][r, 128*ic + o] = W2[128*oc + o, 128*ic + r]
    w2r = nc.declare_dram_parameter("w2r", [NDM, 128, D_MODEL], F16, isOutput=False)
    maskb = nc.declare_dram_parameter("maskb", [128, 16], F32, isOutput=False)
    outT = nc.declare_dram_parameter("outT", [D_MODEL, QPC], F16, isOutput=True)

    W = _widths()

    with tile.TileContext(nc) as tc:
        with (
            tc.tile_pool(name="static", bufs=1) as st,
            tc.tile_pool(name="xvpool", bufs=NKT) as xvp,
            tc.tile_pool(name="ktpool", bufs=NKG) as ktp,
            tc.tile_pool(name="atpool", bufs=1) as atp,
        ):
            qT_sb = st.tile([128, QPC], F16, tag="qT")
            ones_sb = st.tile([128, 1], BF16, tag="ones")
            mask_sb = st.tile([128, 16], F32, tag="mask")
            recip_sb = st.tile([128, QPC], F32, tag="recip")
            nc.vector.memset(ones_sb[:], 1.0)

            # ---- critical small loads first (sync queue order == byte order)
            nc.sync.dma_start(out=mask_sb[:], in_=maskb[:])

            es1 = ExitStack()  # SBUF transients: p1 + xts (freed before p34)
            p1 = es1.enter_context(tc.tile_pool(name="p1", bufs=1))
            wq_sb = p1.tile([128, D_MODEL], F16, tag="wq")
            nc.sync.dma_start(out=wq_sb[:], in_=wqr[:])
            wk_sb = p1.tile([128, D_MODEL], F16, tag="wk")
            nc.sync.dma_start(out=wk_sb[:], in_=wkr[:])
            xq_sb = p1.tile([128, NDM * QPC], F16, tag="xq")
            for qq in range(4):
                nc.sync.dma_start(
                    out=xq_sb[:, 4 * QPC * qq : 4 * QPC * (qq + 1)],
                    in_=xqr[:, 4 * QPC * qq : 4 * QPC * (qq + 1)],
                )

            # ---- xT stream DMAs (one contiguous 1MB DMA per key group) ----
            xts = es1.enter_context(tc.tile_pool(name="xts", bufs=6))
            xts_t = []
            last_xts_dma = None
            for kg in range(NKG):
                t = xts.tile([128, NDM * KG], F16, tag="xts", name=f"xts{kg}")
                last_xts_dma = nc.sync.dma_start(out=t[:], in_=xtp[kg])
                xts_t.append(t)

            # ---- xv loads: hard-ordered AFTER the xT stream (the fused
            # phase is DMA-paced on xtp; sharing bandwidth with xv slows the
            # critical stream 1:1 -- measured) ----
            xv_t = []
            for kt in range(NKT):
                t = xvp.tile([128, D_MODEL], BF16, tag="xv", name=f"xv{kt}")
                d = nc.sync.dma_start(out=t[:], in_=xv[128 * kt : 128 * (kt + 1), :])
                if kt == 0:
                    add_dep_helper(d.ins, last_xts_dma.ins,
                                   reason="xv bytes strictly after xT stream")
                xv_t.append(t)

            # ---- qT projection ----
            with tc.tile_pool(name="psq", bufs=1, space="PSUM") as psqp:
                psq = psqp.tile([128, QPC], F32, tag="psq")
                for ic in range(NDM):
                    nc.tensor.matmul(
                        psq[:],
                        wq_sb[:, 128 * ic : 128 * (ic + 1)],
                        xq_sb[:, QPC * ic : QPC * (ic + 1)],
                        start=(ic == 0), stop=(ic == NDM - 1),
                    )
                nc.vector.tensor_copy(qT_sb[:], psq[:])

            # ---- fused pipeline: kT proj / scores / exp / denom / V[0:4] ----
            es2 = ExitStack()  # PSUM: psv1 (freed mid-way through V chunk waves)
            psv1 = es2.enter_context(
                tc.tile_pool(name="psv1", bufs=4, space="PSUM", side="right")
            )
            pso1 = [
                psv1.tile([128, QPC], F32, tag="pso1", name=f"pso1_{j}")
                for j in range(4)
            ]
            at_t = []
            with (
                tc.tile_pool(name="psk", bufs=2, space="PSUM") as pskp,
                tc.tile_pool(name="pss", bufs=1, space="PSUM") as pssp,
                tc.tile_pool(name="psd", bufs=1, space="PSUM") as psdp,
            ):
                psd = psdp.tile([1, QPC], F32, tag="psd")
                for kg in range(NKG):
                    psk = pskp.tile([128, KG], F32, tag="psk", name=f"psk{kg}")
                    for ic in range(NDM):
                        nc.tensor.matmul(
                            psk[:],
                            wk_sb[:, 128 * ic : 128 * (ic + 1)],
                            xts_t[kg][:, KG * ic : KG * (ic + 1)],
                            start=(ic == 0), stop=(ic == NDM - 1),
                        )
                    ktile = ktp.tile([128, KG], F16, tag="kt", name=f"kt{kg}")
                    nc.vector.tensor_copy(ktile[:], psk[:])

                    for sub in range(KG // 128):
                        kt = kg * (KG // 128) + sub
                        w = W[kt]
                        ps = pssp.tile([128, 512], F32, tag="pss", name=f"pss{kt}")
                        nc.tensor.matmul(
                            ps[:, :w],
                            ktile[:, 128 * sub : 128 * (sub + 1)],
                            qT_sb[:, QPC - w : QPC],
                            start=True, stop=True,
                        )
                        nc.vector.tensor_add(ps[:, :16], ps[:, :16], mask_sb[:])
                        at = atp.tile([128, w], BF16, tag=f"at{kt}")
                        nc.scalar.activation(
                            at[:], ps[:, :w], mybir.ActivationFunctionType.Exp
                        )
                        at_t.append(at)
                        nc.tensor.matmul(
                            psd[0:1, QPC - w : QPC],
                            ones_sb[:],
                            at[:],
                            start=(kt == 0), stop=(kt == NKT - 1),
                        )
                        # V matmul for output chunks 0-3, fused
                        for j in range(4):
                            nc.tensor.matmul(
                                pso1[j][:, QPC - w : QPC],
                                xv_t[kt][:, 128 * j : 128 * (j + 1)],
                                at[:],
                                start=(kt == 0), stop=(kt == NKT - 1),
                            )

                # softmax denominators -> reciprocal (reads psd before the
                # pool closes; broadcast follows outside)
                nc.vector.reciprocal(recip_sb[0:1, :], psd[0:1, :])

            es1.close()  # free p1 + xts SBUF for the aoT / W2 pools
            nc.gpsimd.partition_broadcast(recip_sb[:], recip_sb[0:1, :])

            with tc.tile_pool(name="p34", bufs=1) as p34:
                ao_t = {}

                def v_group(pool, ocs):
                    pso = {
                        oc: pool.tile([128, QPC], F32, tag="pso2", name=f"pso2_{oc}")
                        for oc in ocs
                    }
                    for kt in range(NKT):
                        w = W[kt]
                        for oc in ocs:
                            nc.tensor.matmul(
                                pso[oc][:, QPC - w : QPC],
                                xv_t[kt][:, 128 * oc : 128 * (oc + 1)],
                                at_t[kt][:],
                                start=(kt == 0), stop=(kt == NKT - 1),
                            )
                    for oc in ocs:
                        t = p34.tile([128, QPC], F16, tag=f"ao{oc}")
                        nc.vector.tensor_mul(t[:], pso[oc][:], recip_sb[:])
                        ao_t[oc] = t

                # ---- V matmul for output chunks 4-15, three kt-major 4-chunk
                # groups on explicitly staged PSUM pools (stack allocator:
                # each pool reuses banks of an already-released pool, so
                # group A depends only on the fused pools' release and runs
                # CONCURRENTLY with the xv-paced tail of V[0:4]). ----
                es3 = ExitStack()
                psv2a = es3.enter_context(
                    tc.tile_pool(name="psv2a", bufs=4, space="PSUM")
                )
                v_group(psv2a, range(4, 8))       # banks of psk/pss/psd

                for j in range(4):
                    t = p34.tile([128, QPC], F16, tag=f"ao{j}")
                    # normalize here so fp16 stays in range
                    nc.vector.tensor_mul(t[:], pso1[j][:], recip_sb[:])
                    ao_t[j] = t
                es2.close()                        # release psv1 (banks 4-7)

                with tc.tile_pool(name="psv2b", bufs=4, space="PSUM", side="right") as psv2b:
                    v_group(psv2b, range(8, 12))   # reuses psv1's banks
                    es3.close()                    # release psv2a
                    with tc.tile_pool(name="psv2c", bufs=4, space="PSUM") as psv2c:
                        v_group(psv2c, range(12, NDM))

                # ---- W2: outT = W2T.T @ attn_outT ----
                with (
                    tc.tile_pool(name="w2s", bufs=4) as w2s,
                    tc.tile_pool(name="outs", bufs=4) as outs,
                    tc.tile_pool(name="ps4", bufs=2, space="PSUM") as ps4,
                ):
                    for oc in range(NDM):
                        tw = w2s.tile([128, D_MODEL], F16, tag="w2")
                        nc.sync.dma_start(out=tw[:], in_=w2r[oc])
                        ps = ps4.tile([128, QPC], F32, tag="ps4")
                        for ic in range(NDM):
                            nc.tensor.matmul(
                                ps[:],
                                tw[:, 128 * ic : 128 * (ic + 1)],
                                ao_t[ic][:],
                                start=(ic == 0), stop=(ic == NDM - 1),
                            )
                        t = outs.tile([128, QPC], F16, tag="out")
                        nc.vector.tensor_copy(t[:], ps[:])
                        nc.scalar.dma_start(
                            out=outT[128 * oc : 128 * (oc + 1), :], in_=t[:]
                        )

    nc.compile()
    return nc


def prepare_inputs(x, Wk, Wq, W2):
    """Host-side sharding/layout prep. Returns in_maps for the 8 cores."""
    x = np.asarray(x, dtype=np.float32)
    Wk = np.asarray(Wk, dtype=np.float32)
    Wq = np.asarray(Wq, dtype=np.float32)
    W2 = np.asarray(W2, dtype=np.float32)

    xT16 = np.ascontiguousarray(x.T).astype(np.float16)          # [D, N]
    # xtp[kg, r, KG*ic + n] = xT[128*ic + r, KG*kg + n]
    xtp = np.ascontiguousarray(
        xT16.reshape(NDM, 128, NKG, KG).transpose(2, 1, 0, 3).reshape(NKG, 128, NDM * KG)
    )
    xv16 = x.astype(ml_dtypes.bfloat16)                          # [N, D]

    def pack_chunks(aT, width):
        # aT [D_MODEL, width] -> [128, NDM*width]: out[r, width*ic + c] = aT[128ic+r, c]
        return np.ascontiguousarray(
            aT.reshape(NDM, 128, width).transpose(1, 0, 2).reshape(128, NDM * width)
        )

    wqr = pack_chunks(np.ascontiguousarray(Wq.T).astype(np.float16), D_HEAD)
    wkr = pack_chunks(np.ascontiguousarray(Wk.T).astype(np.float16), D_HEAD)
    # w2r[oc, r, 128*ic + o] = W2T[128ic+r, 128oc+o]
    w2T = np.ascontiguousarray(W2.T).astype(np.float16)
    w2r = np.ascontiguousarray(
        w2T.reshape(NDM, 128, NDM, 128).transpose(2, 1, 0, 3).reshape(NDM, 128, D_MODEL)
    )

    in_maps = []
    for c in range(NCORES):
        xqT = np.ascontiguousarray(x[c::NCORES].T).astype(np.float16)  # [D, QPC]
        xqr_c = pack_chunks(xqT, QPC)
        mask = np.zeros((128, 16), dtype=np.float32)
        j = np.arange(128)[:, None]
        t = np.arange(16)[None, :]
        mask[j > 8 * t + c] = MASK_NEG
        in_maps.append(
            {
                "xqr": xqr_c,
                "xtp": xtp,
                "xv": xv16,
                "wqr": wqr,
                "wkr": wkr,
                "w2r": w2r,
                "maskb": mask,
            }
        )
    return in_maps


def assemble_output(results):
    res = np.stack([np.asarray(results[c]["outT"]).astype(np.float32) for c in range(NCORES)])
    # [c, d, m] -> out[8m+c, d]
    return np.ascontiguousarray(res.transpose(2, 0, 1).reshape(N_CTX, D_MODEL))


_CACHED = {}


def kernel(x, Wk, Wq, W2, _trace=False):
    if "nc" not in _CACHED:
        _CACHED["nc"] = build_program()
    nc = _CACHED["nc"]
    in_maps = prepare_inputs(x, Wk, Wq, W2)
    res = run_bass_kernel_spmd(nc, in_maps, core_ids=list(range(NCORES)), trace=_trace)
    out = assemble_output(res.results)
    if _trace:
        return out, res
    return out


# revision 36
# speedup vs baseline: 1.0695x; 1.0308x over previous
"""Causal attention kernel for TRN2, 8 NeuronCores (SPMD).

Problem:  x[4096,2048] f32; q = x@Wq.T, k = x@Wk.T (d_head=128),
          scores = q@k.T causal-masked, attn = softmax(scores),
          out = (attn @ x) @ W2.T.

Sharding: sequence-parallel over queries with stride-8 interleave:
  core c owns queries {8m+c : m=0..511}.  For key tile kt (128 keys),
  every core has exactly 512-16*kt valid queries -- a contiguous tail
  slice of its query columns -- so the SPMD program is identical on all
  cores (no dynamic control flow, no collectives) and causal work is
  perfectly balanced.

Precision: fp16 inputs for the q/k projections and the score matmul
  (fp32 PSUM accumulation), unnormalized softmax (exp without
  max-subtraction: scores are bounded ~|s|<70 for unit-normal inputs,
  safely inside fp32 exp range), attention weights in bf16 (bf16 has
  fp32 exponent range, needed for exp(s) up to ~1e28), V and W2
  matmuls in bf16/fp16, normalization by the softmax row-sum applied
  at the attn_out eviction (keeps fp16 in range).

Scheduling notes (final, ~226us median on HW; 8-core uniform 221-232us):
  * All input streams are host-packed so every consumer group loads
    with one large contiguous DMA (HWDGE issue costs ~0.6us each; a
    naive per-tile version with 481 issues was issue-bound at 457us).
  * Byte schedule: small projection inputs -> xtp (packed x^T, pacing
    kT -> scores -> exp) -> xv (x natural, pacing the V matmul) -> w2r.
    xv is hard-ordered after the xT stream via an explicit dependency
    on the issuing sequencer: the machine is DMA-bound at ~270 GB/s/core
    through the first two streams, so any byte stealing from xtp slows
    the critical path 1:1 (measured both ways).
  * kT / attnT are split into per-keytile tiles so Tile's dependency
    tracking lets scores/exp/denominator/V pipeline into the DMA
    streams with no whole-tensor barriers.
  * The V matmul for output chunks 0-3 is fused into the score loop
    (PSUM: kT 2 + scores 1 + denom 1 + V 4 = 8 banks); chunks 4-15 run
    as kt-major groups on explicitly staged PSUM pools (stack
    allocator: group A reuses the fused pools' banks and overlaps the
    xv-paced tail of V[0:3]; B/C follow as banks release).  V
    accumulation groups execute start->stop in kt order, so xv must
    stream ascending.
  * Rejected variants (measured): AllGather-sharded kT (8-core 0.5MB
    AllGather costs ~100us here, replication is cheaper), interleaving
    xv into the xtp stream (fused phase is DMA-paced, no slack),
    descending xv (PSUM accumulation groups execute in emission order,
    the whole group waited for xv[0]).
"""

from contextlib import ExitStack

import numpy as np
import ml_dtypes

import concourse.bass as bass
import concourse.bacc as bacc
import concourse.mybir as mybir
import concourse.tile as tile
from concourse.bass_utils import run_bass_kernel_spmd
from concourse.tile_rust import add_dep_helper

N_CTX = 4096
D_MODEL = 2048
D_HEAD = 128
NCORES = 8
QPC = N_CTX // NCORES          # 512 queries per core
NKT = N_CTX // 128             # 32 key tiles
NDM = D_MODEL // 128           # 16 d_model chunks
KG = 128                       # kT projection key-group width
NKG = N_CTX // KG
MASK_NEG = -1.0e30

F16 = mybir.dt.float16
BF16 = mybir.dt.bfloat16
F32 = mybir.dt.float32


def _widths():
    # valid query-column width per key tile (tail slice [512-w : 512] of qT)
    return [QPC - 16 * kt for kt in range(NKT)]


def build_program():
    nc = bacc.Bacc(trn_type="TRN2", target_bir_lowering=False, debug=False)

    # ---- DRAM parameters (identical shapes on all cores; data differs) ----
    # xqr[r, 512*ic + m] = x[8m+c, 128*ic + r]   (own-query columns, packed)
    xqr = nc.declare_dram_parameter("xqr", [128, NDM * QPC], F16, isOutput=False)
    # xtp[kg][r, KG*ic + n] = x[KG*kg + n, 128*ic + r]  (contiguous per-kg tiles)
    xtp = nc.declare_dram_parameter("xtp", [NKG, 128, NDM * KG], F16, isOutput=False)
    # xv = x (natural layout), bf16
    xv = nc.declare_dram_parameter("xv", [N_CTX, D_MODEL], BF16, isOutput=False)
    # wqr[r, 128*ic + h] = Wq[h, 128*ic + r]; same for wkr
    wqr = nc.declare_dram_parameter("wqr", [128, D_MODEL], F16, isOutput=False)
    wkr = nc.declare_dram_parameter("wkr", [128, D_MODEL], F16, isOutput=False)
    # w2r[oc][r, 128*ic + o] = W2[128*oc + o, 128*ic + r]
    w2r = nc.declare_dram_parameter("w2r", [NDM, 128, D_MODEL], F16, isOutput=False)
    maskb = nc.declare_dram_parameter("maskb", [128, 16], F32, isOutput=False)
    outT = nc.declare_dram_parameter("outT", [D_MODEL, QPC], F16, isOutput=True)

    W = _widths()

    with tile.TileContext(nc) as tc:
        with (
            tc.tile_pool(name="static", bufs=1) as st,
            tc.tile_pool(name="xvpool", bufs=NKT) as xvp,
            tc.tile_pool(name="ktpool", bufs=NKG) as ktp,
            tc.tile_pool(name="atpool", bufs=1) as atp,
        ):
            qT_sb = st.tile([128, QPC], F16, tag="qT")
            ones_sb = st.tile([128, 1], BF16, tag="ones")
            mask_sb = st.tile([128, 16], F32, tag="mask")
            recip_sb = st.tile([128, QPC], F32, tag="recip")
            nc.vector.memset(ones_sb[:], 1.0)

            # ---- critical small loads first (sync queue order == byte order)
            nc.sync.dma_start(out=mask_sb[:], in_=maskb[:])

            es1 = ExitStack()  # SBUF transients: p1 + xts (freed before p34)
            p1 = es1.enter_context(tc.tile_pool(name="p1", bufs=1))
            wq_sb = p1.tile([128, D_MODEL], F16, tag="wq")
            nc.sync.dma_start(out=wq_sb[:], in_=wqr[:])
            wk_sb = p1.tile([128, D_MODEL], F16, tag="wk")
            nc.sync.dma_start(out=wk_sb[:], in_=wkr[:])
            xq_sb = p1.tile([128, NDM * QPC], F16, tag="xq")
            for qq in range(4):
                nc.sync.dma_start(
                    out=xq_sb[:, 4 * QPC * qq : 4 * QPC * (qq + 1)],
                    in_=xqr[:, 4 * QPC * qq : 4 * QPC * (qq + 1)],
                )

            # ---- xT stream DMAs (one contiguous 1MB DMA per key group) ----
            xts = es1.enter_context(tc.tile_pool(name="xts", bufs=6))
            xts_t = []
            last_xts_dma = None
            for kg in range(NKG):
                t = xts.tile([128, NDM * KG], F16, tag="xts", name=f"xts{kg}")
                last_xts_dma = nc.sync.dma_start(out=t[:], in_=xtp[kg])
                xts_t.append(t)

            # ---- xv loads: hard-ordered AFTER the xT stream (the fused
            # phase is DMA-paced on xtp; sharing bandwidth with xv slows the
            # critical stream 1:1 -- measured) ----
            xv_t = []
            for kt in range(NKT):
                t = xvp.tile([128, D_MODEL], BF16, tag="xv", name=f"xv{kt}")
                d = nc.sync.dma_start(out=t[:], in_=xv[128 * kt : 128 * (kt + 1), :])
                if kt == 0:
                    add_dep_helper(d.ins, last_xts_dma.ins,
                                   reason="xv bytes strictly after xT stream")
                xv_t.append(t)

            # ---- qT projection ----
            with tc.tile_pool(name="psq", bufs=1, space="PSUM") as psqp:
                psq = psqp.tile([128, QPC], F32, tag="psq")
                for ic in range(NDM):
                    nc.tensor.matmul(
                        psq[:],
                        wq_sb[:, 128 * ic : 128 * (ic + 1)],
                        xq_sb[:, QPC * ic : QPC * (ic + 1)],
                        start=(ic == 0), stop=(ic == NDM - 1),
                    )
                nc.vector.tensor_copy(qT_sb[:], psq[:])

            # ---- fused pipeline: kT proj / scores / exp / denom / V[0:4] ----
            es2 = ExitStack()  # PSUM: psv1 (freed mid-way through V chunk waves)
            psv1 = es2.enter_context(
                tc.tile_pool(name="psv1", bufs=4, space="PSUM", side="right")
            )
            pso1 = [
                psv1.tile([128, QPC], F32, tag="pso1", name=f"pso1_{j}")
                for j in range(4)
            ]
            at_t = []
            with (
                tc.tile_pool(name="psk", bufs=2, space="PSUM") as pskp,
                tc.tile_pool(name="pss", bufs=1, space="PSUM") as pssp,
                tc.tile_pool(name="psd", bufs=1, space="PSUM") as psdp,
            ):
                psd = psdp.tile([1, QPC], F32, tag="psd")
                for kg in range(NKG):
                    psk = pskp.tile([128, KG], F32, tag="psk", name=f"psk{kg}")
                    for ic in range(NDM):
                        nc.tensor.matmul(
                            psk[:],
                            wk_sb[:, 128 * ic : 128 * (ic + 1)],
                            xts_t[kg][:, KG * ic : KG * (ic + 1)],
                            start=(ic == 0), stop=(ic == NDM - 1),
                        )
                    ktile = ktp.tile([128, KG], F16, tag="kt", name=f"kt{kg}")
                    nc.vector.tensor_copy(ktile[:], psk[:])

                    for sub in range(KG // 128):
                        kt = kg * (KG // 128) + sub
                        w = W[kt]
                        ps = pssp.tile([128, 512], F32, tag="pss", name=f"pss{kt}")
                        nc.tensor.matmul(
                            ps[:, :w],
                            ktile[:, 128 * sub : 128 * (sub + 1)],
                            qT_sb[:, QPC - w : QPC],
                            start=True, stop=True,
                        )
                        nc.vector.tensor_add(ps[:, :16], ps[:, :16], mask_sb[:])
                        at = atp.tile([128, w], BF16, tag=f"at{kt}")
                        nc.scalar.activation(
                            at[:], ps[:, :w], mybir.ActivationFunctionType.Exp
                        )
                        at_t.append(at)
                        nc.tensor.matmul(
                            psd[0:1, QPC - w : QPC],
                            ones_sb[:],
                            at[:],
                            start=(kt == 0), stop=(kt == NKT - 1),
                        )
                        # V matmul for output chunks 0-3, fused
                        # (first key half only; the early eviction at xv[15]
                        # frees all 8 banks for the oc 8-15 full streams)
                        if kt < NKT // 2:
                            for j in range(4):
                                nc.tensor.matmul(
                                    pso1[j][:, QPC - w : QPC],
                                    xv_t[kt][:, 128 * j : 128 * (j + 1)],
                                    at[:],
                                    start=(kt == 0), stop=(kt == NKT // 2 - 1),
                                )

                # softmax denominators -> reciprocal (reads psd before the
                # pool closes; broadcast follows outside)
                nc.vector.reciprocal(recip_sb[0:1, :], psd[0:1, :])

            es1.close()  # free p1 + xts SBUF for the aoT / W2 pools
            nc.gpsimd.partition_broadcast(recip_sb[:], recip_sb[0:1, :])

            with tc.tile_pool(name="p34", bufs=1) as p34:
                ao_t = {}

                # ---- oc 4-7, first key half (kt 0-15): runs concurrently
                # with the fused V[0:3] streams on the banks freed by the
                # fused pools; both finish when xv[15] lands ----
                HK = NKT // 2
                with tc.tile_pool(name="g2", bufs=4, space="PSUM") as g2p:
                    g2 = {
                        oc: g2p.tile([128, QPC], F32, tag="g2", name=f"g2_{oc}")
                        for oc in range(4, 8)
                    }
                    for kt in range(HK):
                        w = W[kt]
                        for oc in range(4, 8):
                            nc.tensor.matmul(
                                g2[oc][:, QPC - w : QPC],
                                xv_t[kt][:, 128 * oc : 128 * (oc + 1)],
                                at_t[kt][:],
                                start=(kt == 0), stop=(kt == HK - 1),
                            )
                    # early normalized evictions for oc 0-7 (partial over the
                    # first key half; exact for queries m<256 by causality)
                    for j in range(4):
                        t = p34.tile([128, QPC], F16, tag=f"ao{j}")
                        nc.vector.tensor_mul(t[:], pso1[j][:], recip_sb[:])
                        ao_t[j] = t
                    es2.close()  # release the fused V banks
                    for oc in range(4, 8):
                        t = p34.tile([128, QPC], F16, tag=f"ao{oc}")
                        nc.vector.tensor_mul(t[:], g2[oc][:], recip_sb[:])
                        ao_t[oc] = t

                # ---- oc 8-15, FULL key range: 8 concurrent streams on the
                # freed banks; kt 0-15 runs dense from resident xv while the
                # xv tail streams in ----
                with (
                    tc.tile_pool(name="bcL", bufs=4, space="PSUM") as bcL,
                    tc.tile_pool(name="bcR", bufs=4, space="PSUM", side="right") as bcR,
                ):
                    bc = {}
                    for i, oc in enumerate(range(8, NDM)):
                        pool = bcL if i < 4 else bcR
                        bc[oc] = pool.tile(
                            [128, QPC], F32, tag="bc", name=f"bc_{oc}"
                        )
                    for kt in range(NKT):
                        w = W[kt]
                        for oc in range(8, NDM):
                            nc.tensor.matmul(
                                bc[oc][:, QPC - w : QPC],
                                xv_t[kt][:, 128 * oc : 128 * (oc + 1)],
                                at_t[kt][:],
                                start=(kt == 0), stop=(kt == NKT - 1),
                            )
                    for oc in range(8, NDM):
                        t = p34.tile([128, QPC], F16, tag=f"ao{oc}")
                        nc.vector.tensor_mul(t[:], bc[oc][:], recip_sb[:])
                        ao_t[oc] = t

                # ---- oc 0-7, second key half (kt 16-31, queries [256:512]
                # only): dense from resident xv, merged into the early aos ----
                with (
                    tc.tile_pool(name="g4", bufs=4, space="PSUM") as g4p,
                    tc.tile_pool(name="tmr", bufs=4) as tmr,
                ):
                    for ocs in (range(0, 4), range(4, 8)):
                        g4 = {
                            oc: g4p.tile(
                                [128, QPC // 2], F32, tag="g4", name=f"g4_{oc}"
                            )
                            for oc in ocs
                        }
                        for kt in range(HK, NKT):
                            w = W[kt]
                            for oc in ocs:
                                nc.tensor.matmul(
                                    g4[oc][:, QPC // 2 - w : QPC // 2],
                                    xv_t[kt][:, 128 * oc : 128 * (oc + 1)],
                                    at_t[kt][:],
                                    start=(kt == HK), stop=(kt == NKT - 1),
                                )
                        for oc in ocs:
                            tm = tmr.tile([128, QPC // 2], F16, tag="tm")
                            nc.vector.tensor_mul(
                                tm[:], g4[oc][:], recip_sb[:, QPC // 2 :]
                            )
                            nc.vector.tensor_add(
                                ao_t[oc][:, QPC // 2 :],
                                ao_t[oc][:, QPC // 2 :],
                                tm[:],
                            )

                    # ---- W2: outT = W2T.T @ attn_outT.  ic order 8..15 first
                    # (those aos finish earliest), 0..7 after the merges ----
                    with (
                        tc.tile_pool(name="w2s", bufs=4) as w2s,
                        tc.tile_pool(name="outs", bufs=4) as outs,
                        tc.tile_pool(name="ps4", bufs=4, space="PSUM", side="right") as ps4,
                    ):
                        ic_order = list(range(8, NDM)) + list(range(0, 8))
                        for oc in range(NDM):
                            tw = w2s.tile([128, D_MODEL], F16, tag="w2")
                            nc.sync.dma_start(out=tw[:], in_=w2r[oc])
                            ps = ps4.tile([128, QPC], F32, tag="ps4")
                            for i, ic in enumerate(ic_order):
                                nc.tensor.matmul(
                                    ps[:],
                                    tw[:, 128 * ic : 128 * (ic + 1)],
                                    ao_t[ic][:],
                                    start=(i == 0), stop=(i == NDM - 1),
                                )
                            t = outs.tile([128, QPC], F16, tag="out")
                            nc.vector.tensor_copy(t[:], ps[:])
                            nc.scalar.dma_start(
                                out=outT[128 * oc : 128 * (oc + 1), :], in_=t[:]
                            )

    nc.compile()
    return nc


def prepare_inputs(x, Wk, Wq, W2):
    """Host-side sharding/layout prep. Returns in_maps for the 8 cores."""
    x = np.asarray(x, dtype=np.float32)
    Wk = np.asarray(Wk, dtype=np.float32)
    Wq = np.asarray(Wq, dtype=np.float32)
    W2 = np.asarray(W2, dtype=np.float32)

    xT16 = np.ascontiguousarray(x.T).astype(np.float16)          # [D, N]
    # xtp[kg, r, KG*ic + n] = xT[128*ic + r, KG*kg + n]
    xtp = np.ascontiguousarray(
        xT16.reshape(NDM, 128, NKG, KG).transpose(2, 1, 0, 3).reshape(NKG, 128, NDM * KG)
    )
    xv16 = x.astype(ml_dtypes.bfloat16)                          # [N, D]

    def pack_chunks(aT, width):
        # aT [D_MODEL, width] -> [128, NDM*width]: out[r, width*ic + c] = aT[128ic+r, c]
        return np.ascontiguousarray(
            aT.reshape(NDM, 128, width).transpose(1, 0, 2).reshape(128, NDM * width)
        )

    wqr = pack_chunks(np.ascontiguousarray(Wq.T).astype(np.float16), D_HEAD)
    wkr = pack_chunks(np.ascontiguousarray(Wk.T).astype(np.float16), D_HEAD)
    # w2r[oc, r, 128*ic + o] = W2T[128ic+r, 128oc+o]
    w2T = np.ascontiguousarray(W2.T).astype(np.float16)
    w2r = np.ascontiguousarray(
        w2T.reshape(NDM, 128, NDM, 128).transpose(2, 1, 0, 3).reshape(NDM, 128, D_MODEL)
    )

    in_maps = []
    for c in range(NCORES):
        xqT = np.ascontiguousarray(x[c::NCORES].T).astype(np.float16)  # [D, QPC]
        xqr_c = pack_chunks(xqT, QPC)
        mask = np.zeros((128, 16), dtype=np.float32)
        j = np.arange(128)[:, None]
        t = np.arange(16)[None, :]
        mask[j > 8 * t + c] = MASK_NEG
        in_maps.append(
            {
                "xqr": xqr_c,
                "xtp": xtp,
                "xv": xv16,
                "wqr": wqr,
                "wkr": wkr,
                "w2r": w2r,
                "maskb": mask,
            }
        )
    return in_maps


def assemble_output(results):
    res = np.stack([np.asarray(results[c]["outT"]).astype(np.float32) for c in range(NCORES)])
    # [c, d, m] -> out[8m+c, d]
    return np.ascontiguousarray(res.transpose(2, 0, 1).reshape(N_CTX, D_MODEL))


_CACHED = {}


def kernel(x, Wk, Wq, W2, _trace=False):
    if "nc" not in _CACHED:
        _CACHED["nc"] = build_program()
    nc = _CACHED["nc"]
    in_maps = prepare_inputs(x, Wk, Wq, W2)
    res = run_bass_kernel_spmd(nc, in_maps, core_ids=list(range(NCORES)), trace=_trace)
    out = assemble_output(res.results)
    if _trace:
        return out, res
    return out
